# revision 1
# baseline (speedup 1.0000x reference)
"""Trainium2 Bass kernel for nn_Block_31722628448427 (dense transformer block
with multi-head latent attention + gated FFN).

Sharding over 8 NeuronCores: core c handles batch b = c//2 and sequence half
hf = c%2 (2048 tokens).  Attention k/v/scores/o are computed for the local
half only; the pair of cores combines unnormalized o and softmax partition
sums z with a tiny pair-wise DRAM AllReduce, then each core normalizes,
out-projects, expands latents->seq for its half and runs LN2 + the FFN.

All heavy matmuls run in fp8e4 (e4m3) with DoubleRow perf mode (2x PE rate)
and fp32 PSUM accumulation.  Power-of-two scale factors keep fp8 operands
inside the +-240 e4m3 range and are folded back in copy-out activations.
x and the residual stream are pre-scaled by 2048 host-side (LN is scale
invariant; the host divides the output back) so the FFN epilogue is a plain
add.  Matmul loops are ordered so each stationary (weight) tile is reused
across several moving-side matmuls - weight loads dominate otherwise.
"""
import contextlib

import numpy as np
import ml_dtypes

import bass_rust
import concourse.bass as bass
import concourse.tile as tile
from concourse import mybir
from concourse.masks import make_identity

BF16 = ml_dtypes.bfloat16
F8NP = ml_dtypes.float8_e4m3
F32 = mybir.dt.float32
BF = mybir.dt.bfloat16
F8 = mybir.dt.float8e4
AF = mybir.ActivationFunctionType
OP = mybir.AluOpType
DR = mybir.MatmulPerfMode.DoubleRow

B, S, E, H, HD, LD, NL = 4, 4096, 1024, 16, 64, 256, 64
EPS = 1e-5
P = 128
NCORES = 8

# power-of-two fp8 scale plan
XS = 4.0        # LN outputs (xn, h2) stored as 4*x
WS = 8.0        # wk/wv/w1/w2 stored as 8*w  -> psum carries 32*true
QS = 16.0       # q stored as 16*q
W3S = 64.0      # w3 stored as 64*w3
RS = 2048.0     # x / residual pre-scale (folded out host-side)


# --------------------------------------------------------------------------
# walrus on this toolchain accepts at most ONE embedded sync-wait per
# instruction; Tile emits more at cross-engine joins.  Spill extras onto
# standalone same-engine NoOps placed immediately before the instruction.
def _spill_extra_waits(nc):
    counter = 0
    for f in nc.m.functions:
        for bb in f.blocks:
            new_list = []
            changed = False
            for inst in bb.instructions:
                si = inst.sync_info
                waits = list(si.on_wait) if si is not None else []
                if len(waits) > 1:
                    for w in waits[:-1]:
                        nop = mybir.InstNoOp(name=f"wspill_{counter}", ins=[], outs=[])
                        counter += 1
                        nop.engine = inst.engine
                        nop.sync_info = bass_rust.SyncInfo(on_wait=[w], on_update=[])
                        new_list.append(nop)
                    inst.sync_info = bass_rust.SyncInfo(
                        on_wait=waits[-1:], on_update=list(si.on_update)
                    )
                    changed = True
                new_list.append(inst)
            if changed:
                bb.instructions = new_list


# --------------------------------------------------------------------------
def build_program(nc, seq=S, silu_via_sigmoid=False, skip_cc=False, phases="ABC"):
    """Emit the per-core program.  `seq` lets tests build a smaller version."""
    HSEQ = seq // 2          # this core's token count
    ST = HSEQ // P           # 128-token tiles (16)
    NSC = HSEQ // 512        # 512-wide score chunks (4)
    NCH = max(HSEQ // 512, 1)  # FFN chunks
    CH = HSEQ // NCH         # tokens per FFN chunk (512)
    HP = H // 2              # head pairs (8)
    ET = E // P              # 8 e-tiles
    EP = ET // 2             # e-tile pairs for DoubleRow (4)
    FT = 4 * E // P          # 32 hidden tiles

    dram = lambda name, shape, dt, kind="ExternalInput", **kw: nc.dram_tensor(
        name, shape, dt, kind=kind, **kw
    )
    x_d = dram("x_own", [HSEQ, E], BF)
    wk_d = dram("wk", [P, ET, H * HD], F8)
    wv_d = dram("wv", [P, ET, H * HD], F8)
    wq_d = dram("wq", [P, H * (LD // P) * HD], BF)
    lat_d = dram("lat", [P, H * (LD // P) * NL], BF)
    wo_d = dram("wo", [HD, H, E], BF)
    wproj_d = dram("wproj", [NL, HSEQ], BF)
    w1_d = dram("w1", [FT // 4, P, 4, ET, P], F8)
    w2_d = dram("w2", [FT // 4, P, 4, ET, P], F8)
    w3_d = dram("w3", [P, FT, E], F8)
    out_d = dram("out", [HSEQ, E], BF, kind="ExternalOutput")
    cc_d0 = dram("cc_part0", [P, HP * (NL + 1)], BF, kind="Internal")
    ccr_d0 = dram("cc_sum0", [P, HP * (NL + 1)], BF, kind="Internal")

    with tile.TileContext(nc) as tc, contextlib.ExitStack() as ctx:
        const = ctx.enter_context(tc.tile_pool(name="const", bufs=1))
        longp = ctx.enter_context(tc.tile_pool(name="longp", bufs=1))

        ident = const.tile([P, P], BF)
        make_identity(nc, ident)
        # x and xres are pre-scaled by 2048 host-side; LN is scale-invariant
        # but the eps bias must be scaled by 2048^2/16.
        epsS_t = const.tile([P, 1], F32)
        nc.vector.memset(epsS_t, EPS * (RS ** 2) / 16.0)

        wproj_sb = longp.tile([NL, HSEQ], BF)
        nc.sync.dma_start(out=wproj_sb, in_=wproj_d.ap())
        attn_sb = longp.tile([NL, E], BF)
        # qT pair-stacked fp8 (=16*q): partitions j*HD:(j+1)*HD hold head
        # 2*hp+j's q.T
        qT_sb = longp.tile([P, HP * NL], F8)

        # ---------------- phase A: LN1, q/k/v projections ----------------
        ab_ctx = contextlib.ExitStack()
        kvx = ab_ctx.enter_context(tc.tile_pool(name="kvx", bufs=1))
        xnT = kvx.tile([P, ET, HSEQ], F8)          # 4*ln1(x) transposed
        kT_sb = kvx.tile([P, HP, HSEQ], F8)        # 32*k pair-stacked
        v_sb = kvx.tile([P, ST, H * HD], BF)       # 32*v
        wo_sb = kvx.tile([HD, H, E], BF)

        with contextlib.ExitStack() as actx:
            xst = actx.enter_context(tc.tile_pool(name="xst", bufs=4))
            # prefetch the first token tiles so LN1 starts immediately
            x_pre = []
            for tt in range(3):
                x_t = xst.tile([P, E], BF, tag="x")
                nc.sync.dma_start(out=x_t, in_=x_d.ap()[tt * P:(tt + 1) * P, :])
                x_pre.append(x_t)
            # qT first, in a short-lived innermost pool
            small_ctx = contextlib.ExitStack()
            small = small_ctx.enter_context(tc.tile_pool(name="small", bufs=1))
            psQ_ctx = contextlib.ExitStack()
            psQ = psQ_ctx.enter_context(tc.tile_pool(name="psQ", bufs=2, space="PSUM"))
            wq_sb = small.tile([P, H * (LD // P) * HD], BF)
            lat_sb = small.tile([P, H * (LD // P) * NL], BF)
            qn = H * (LD // P) * HD
            for q2 in range(2):
                nc.sync.dma_start(
                    out=wq_sb[:, q2 * qn // 2:(q2 + 1) * qn // 2],
                    in_=wq_d.ap()[:, q2 * qn // 2:(q2 + 1) * qn // 2])
                nc.sync.dma_start(
                    out=lat_sb[:, q2 * qn // 2:(q2 + 1) * qn // 2],
                    in_=lat_d.ap()[:, q2 * qn // 2:(q2 + 1) * qn // 2])

            # qT[h] = Wq[h].T @ latT[h] -> [HD, NL], fp8 out 16*q.
            # PE outputs land at base partition 0; odd heads reach partitions
            # 64..127 via an SBUF->SBUF DMA shift.
            for h in range(H):
                hp, j = h // 2, h % 2
                ps_q = psQ.tile([HD, NL], F32, tag="psq")
                for kt in range(LD // P):
                    iq = (h * (LD // P) + kt) * HD
                    il = (h * (LD // P) + kt) * NL
                    nc.tensor.matmul(
                        ps_q,
                        wq_sb[:, iq:iq + HD],
                        lat_sb[:, il:il + NL],
                        start=(kt == 0), stop=(kt == LD // P - 1),
                    )
                if j == 0:
                    nc.scalar.activation(
                        qT_sb[0:HD, hp * NL:(hp + 1) * NL], ps_q, AF.Copy,
                        scale=QS,
                    )
                else:
                    q_tmp = small.tile([HD, NL], F8, tag="qtmp")
                    nc.scalar.activation(q_tmp, ps_q, AF.Copy, scale=QS)
                    nc.sync.dma_start(
                        out=qT_sb[HD:P, hp * NL:(hp + 1) * NL], in_=q_tmp
                    )
            psQ_ctx.close()
            small_ctx.close()

            stat = actx.enter_context(tc.tile_pool(name="stat", bufs=4))
            stage = actx.enter_context(tc.tile_pool(name="stage", bufs=3))
            wv_p = actx.enter_context(tc.tile_pool(name="wv", bufs=1))
            psT = actx.enter_context(tc.tile_pool(name="psT", bufs=2, space="PSUM"))
            psV = actx.enter_context(tc.tile_pool(name="psV", bufs=2, space="PSUM"))
            psK = actx.enter_context(tc.tile_pool(name="psK", bufs=2, space="PSUM"))

            wv_sb = wv_p.tile([P, ET, H * HD], F8)
            wk_sb = wv_p.tile([P, ET, H * HD], F8)
            for et in range(ET):
                nc.gpsimd.dma_start(out=wv_sb[:, et, :], in_=wv_d.ap()[:, et, :])

            # LN1 + transpose + v-projection, one 128-token tile at a time
            for tt in range(ST):
                if tt < 3:
                    x_t = x_pre[tt]
                else:
                    x_t = xst.tile([P, E], BF, tag="x")
                    nc.sync.dma_start(out=x_t, in_=x_d.ap()[tt * P:(tt + 1) * P, :])
                if tt == max(ST - 8, ST // 2):
                    # wk loads overlap the LN1/v loop tail
                    for et in range(ET):
                        nc.gpsimd.dma_start(out=wk_sb[:, et, :],
                                            in_=wk_d.ap()[:, et, :])
                st_t = stat.tile([P, 2, 6], F32, tag="st")
                xg = x_t.rearrange("p (g d) -> p g d", g=2)
                for g in range(2):
                    nc.vector.bn_stats(out=st_t[:, g, :], in_=xg[:, g, :])
                mv = stat.tile([P, 2], F32, tag="mv")
                nc.vector.bn_aggr(out=mv, in_=st_t)
                # std/4 = sqrt(var/16 + eps/16); rstd4 = 4/std
                std = stat.tile([P, 1], F32, tag="std")
                nc.scalar.activation(std, mv[:, 1:2], AF.Sqrt, bias=epsS_t,
                                     scale=1.0 / 16.0)
                rstd4 = stat.tile([P, 1], F32, tag="rstd")
                nc.vector.reciprocal(rstd4, std)
                xn_t = xst.tile([P, E], BF, tag="xn")       # 4*ln1(x)
                nc.vector.tensor_scalar(
                    out=xn_t, in0=x_t, scalar1=mv[:, 0:1], scalar2=rstd4,
                    op0=OP.subtract, op1=OP.mult,
                )
                # transpose into xnT[:, :, tt*P : (tt+1)*P] (fp8 at copy-out)
                for eh in range(ET // 4):
                    ps_t = psT.tile([P, 4 * P], BF, tag="pst")
                    for j in range(4):
                        et = eh * 4 + j
                        nc.tensor.transpose(
                            ps_t[:, j * P:(j + 1) * P],
                            xn_t[:, et * P:(et + 1) * P], ident,
                        )
                    nc.scalar.activation(
                        xnT[:, eh * 4:(eh + 1) * 4, tt * P:(tt + 1) * P],
                        ps_t.rearrange("p (a b) -> p a b", a=4), AF.Copy,
                    )
                # v projection (DoubleRow): out 64-token halves
                for th in range(2):
                    ps_v = psV.tile([64, H * HD], F32, tag="pv")
                    for n2 in range(2):
                        for ep in range(EP):
                            nc.tensor.matmul(
                                ps_v[:, n2 * 512:(n2 + 1) * 512],
                                xnT[:, 2 * ep:2 * ep + 2,
                                    tt * P + th * 64:tt * P + th * 64 + 64],
                                wv_sb[:, 2 * ep:2 * ep + 2,
                                      n2 * 512:(n2 + 1) * 512],
                                start=(ep == 0), stop=(ep == EP - 1),
                                perf_mode=DR,
                            )
                    if th == 0:
                        if tt % 2 == 0:
                            nc.vector.tensor_copy(v_sb[0:64, tt, :], ps_v)
                        else:
                            nc.scalar.activation(v_sb[0:64, tt, :], ps_v, AF.Copy)
                    else:
                        if tt % 2 == 0:
                            v_tmp = stage.tile([64, 2, H * HD], BF, tag="vtmp")
                            nc.scalar.activation(v_tmp[:, 0, :], ps_v, AF.Copy)
                        else:
                            nc.vector.tensor_copy(v_tmp[:, 1, :], ps_v)
                            nc.gpsimd.dma_start(
                                out=v_sb[64:P, tt - 1:tt + 1, :], in_=v_tmp)

            # kT projection (DoubleRow) -> kT_sb pair-stacked.  sc pairs
            # share the stationary wk slice so its weight load is reused.
            for hp in range(HP):
                k_tmp = stage.tile([64, NSC, 512], F8, tag="ktmp")
                for j in range(2):
                    for grp in [list(range(i, min(i + 2, NSC)))
                                for i in range(0, NSC, 2)]:
                        ps_g = []
                        for _ in grp:
                            ps_k = psK.tile([64, 512], F32, tag="pk")
                            ps_g.append(ps_k)
                        for ep in range(EP):
                            for gi, sc in enumerate(grp):
                                nc.tensor.matmul(
                                    ps_g[gi],
                                    wk_sb[:, 2 * ep:2 * ep + 2,
                                          hp * P + j * 64:hp * P + j * 64 + 64],
                                    xnT[:, 2 * ep:2 * ep + 2,
                                        sc * 512:(sc + 1) * 512],
                                    start=(ep == 0), stop=(ep == EP - 1),
                                    perf_mode=DR,
                                )
                        for gi, sc in enumerate(grp):
                            if j == 0:
                                nc.vector.tensor_copy(
                                    kT_sb[0:64, hp, sc * 512:(sc + 1) * 512], ps_g[gi]
                                )
                            else:
                                nc.scalar.activation(k_tmp[:, sc, :], ps_g[gi], AF.Copy)
                nc.gpsimd.dma_start(
                    out=kT_sb[64:P, hp, :],
                    in_=k_tmp.rearrange("p a b -> p (a b)"),
                )
            for h4 in range(H // 4):
                nc.gpsimd.dma_start(out=wo_sb[:, h4 * 4:(h4 + 1) * 4, :],
                                    in_=wo_d.ap()[:, h4 * 4:(h4 + 1) * 4, :])

        if phases == "A":
            ab_ctx.close()
            with tc.tile_pool(name="dbg", bufs=3) as dbg:
                for tt in range(ST):
                    d_t = dbg.tile([P, E], BF, tag="d")
                    nc.sync.dma_start(out=d_t, in_=x_d.ap()[tt * P:(tt + 1) * P, :])
                    nc.sync.dma_start(out=out_d.ap()[tt * P:(tt + 1) * P, :], in_=d_t)
            return nc

        # ---------------- phase B: latent attention (local half) ----------
        with contextlib.ExitStack() as bctx:
            att = bctx.enter_context(tc.tile_pool(name="att", bufs=2))
            att2 = bctx.enter_context(tc.tile_pool(name="att2", bufs=2))
            qo = bctx.enter_context(tc.tile_pool(name="qo", bufs=1))
            psS = bctx.enter_context(tc.tile_pool(name="psS", bufs=1, space="PSUM"))
            psO = bctx.enter_context(tc.tile_pool(name="psO", bufs=1, space="PSUM"))

            cc_sb = qo.tile([P, HP, NL + 1], BF)    # o partial + 32*z partial

            for hp in range(HP):
                w_t = att.tile([P, HSEQ], BF, tag="w")      # exp(logits)
                zp = att.tile([P, NSC], F32, tag="zp")
                ps_l = []
                for sc in range(NSC):
                    ps_s = psS.tile([P, 512], F32, tag=f"ps{sc}")
                    ps_l.append(ps_s)
                # j-outer so each qT weight load serves all sc chunks
                for j in range(2):
                    for sc in range(NSC):
                        nc.tensor.matmul(
                            ps_l[sc][j * NL:(j + 1) * NL, :],
                            qT_sb[j * HD:(j + 1) * HD, hp * NL:(hp + 1) * NL],
                            kT_sb[j * HD:(j + 1) * HD, hp, sc * 512:(sc + 1) * 512],
                            start=True, stop=True,
                        )
                for sc in range(NSC):
                    # logits_ps = 512*qk ; true logit = qk/8
                    nc.scalar.activation(
                        w_t[:, sc * 512:(sc + 1) * 512], ps_l[sc], AF.Exp,
                        scale=1.0 / 4096.0,
                        accum_out=zp[:, sc:sc + 1],
                    )
                z_t = att.tile([P, 1], F32, tag="z")
                nc.vector.tensor_reduce(z_t, zp, axis=mybir.AxisListType.X, op=OP.add)
                # cc z slot holds 32*z so o_sum/(32 z_sum) = true o/z / 32
                nc.vector.tensor_scalar_mul(cc_sb[:, hp, NL:NL + 1], z_t, 32.0)
                # transpose exp-weights with the XBAR DMA engine (2-byte dtype)
                wT_t = att2.tile([P, ST, P], BF, tag="wT")
                nc.sync.dma_start_transpose(wT_t, w_t)
                # lhsT = wT so out blocks are [latent, hd]; z is per-latent
                ps_o = psO.tile([P, P], F32, tag="po")
                for st_i in range(ST):
                    nc.tensor.matmul(
                        ps_o, wT_t[:, st_i, :], v_sb[:, st_i, hp * P:(hp + 1) * P],
                        start=(st_i == 0), stop=(st_i == ST - 1),
                    )
                # diagonal blocks hold the two heads' partial o (32*o)
                nc.vector.tensor_copy(cc_sb[0:64, hp, 0:NL], ps_o[0:64, 0:NL])
                nc.vector.tensor_copy(cc_sb[64:P, hp, 0:NL], ps_o[64:P, NL:2 * NL])

            # pair-wise combine of (32*o, 32*z) partials
            nc.sync.dma_start(out=cc_d0.ap(),
                              in_=cc_sb.rearrange("p a b -> p (a b)"))
            if skip_cc:
                nc.gpsimd.dma_start(out=ccr_d0.ap(), in_=cc_d0.ap())
            else:
                nc.gpsimd.collective_compute(
                    "AllReduce", OP.add,
                    replica_groups=[[0, 1], [2, 3], [4, 5], [6, 7]],
                    ins=[cc_d0.ap().opt()], outs=[ccr_d0.ap().opt()],
                )
            ccr_sb = qo.tile([P, HP, NL + 1], BF)
            nc.sync.dma_start(out=ccr_sb.rearrange("p a b -> p (a b)"),
                              in_=ccr_d0.ap())

            rz = qo.tile([P, HP], F32)
            nc.vector.reciprocal(rz, ccr_sb[:, :, NL])
            o_n = qo.tile([P, HP, NL], BF)          # normalized o, latent rows
            for hp in range(HP):
                nc.vector.tensor_scalar_mul(
                    o_n[:, hp, :], ccr_sb[:, hp, 0:NL], rz[:, hp:hp + 1]
                )
            o_hi = qo.tile([64, HP, NL], BF)
            nc.sync.dma_start(out=o_hi, in_=o_n[64:P, :, :])

            # oT[h] = o[h].T -> [HD, NL] at partitions 0..63
            oT_flat = qo.tile([HD, H, NL], BF)
            psF = bctx.enter_context(tc.tile_pool(name="psF", bufs=1, space="PSUM"))
            for h in range(H):
                hp, j = h // 2, h % 2
                src = o_n[0:64, hp, :] if j == 0 else o_hi[:, hp, :]
                ps_tq = psF.tile([NL, HD], BF, tag="ptq")
                nc.tensor.transpose(ps_tq, src, ident[0:64, 0:64])
                nc.vector.tensor_copy(oT_flat[:, h, :], ps_tq)

            # out-projection: attn[NL, E] = sum_h oT[h].T @ Wo[h]
            ps_at0 = psF.tile([NL, 512], F32, tag="pat0")
            ps_at1 = psF.tile([NL, 512], F32, tag="pat1")
            for n2 in range(2):
                ps_at = ps_at0 if n2 == 0 else ps_at1
                for h in range(H):
                    nc.tensor.matmul(
                        ps_at,
                        oT_flat[:, h, :],
                        wo_sb[:, h, n2 * 512:(n2 + 1) * 512],
                        start=(h == 0), stop=(h == H - 1),
                    )
                # attn_sb carries 2048*attn for the pre-scaled residual path
                nc.vector.tensor_scalar_mul(
                    attn_sb[:, n2 * 512:(n2 + 1) * 512], ps_at, RS)
        ab_ctx.close()

        if phases == "AB":
            with tc.tile_pool(name="dbg", bufs=3) as dbg:
                a_t = dbg.tile([NL, E], BF)
                nc.vector.tensor_copy(a_t, attn_sb)
                nc.sync.dma_start(out=out_d.ap()[0:NL, :], in_=a_t)
                for tt in range(1, ST):
                    d_t = dbg.tile([P, E], BF, tag="d")
                    nc.sync.dma_start(out=d_t, in_=x_d.ap()[tt * P:(tt + 1) * P, :])
                    nc.sync.dma_start(out=out_d.ap()[tt * P:(tt + 1) * P, :], in_=d_t)
            return nc

        # ---------------- phase C: latent->seq, LN2, FFN ----------------
        with contextlib.ExitStack() as fctx:
            cres = fctx.enter_context(tc.tile_pool(name="cres", bufs=1))
            xres = cres.tile([P, ST, E], BF)        # 2048*(x + attn_exp)
            h2T = cres.tile([P, ET, HSEQ], F8)      # 4*ln2(xres) transposed

            with contextlib.ExitStack() as c1ctx:
                x2st = c1ctx.enter_context(tc.tile_pool(name="x2st", bufs=3))
                stat2 = c1ctx.enter_context(tc.tile_pool(name="stat2", bufs=4))
                psSQ = c1ctx.enter_context(tc.tile_pool(name="psSQ", bufs=1, space="PSUM"))
                psT3 = c1ctx.enter_context(tc.tile_pool(name="psT3", bufs=1, space="PSUM"))
                for tt in range(ST):
                    xh_t = x2st.tile([P, E], BF, tag="xh")
                    nc.sync.dma_start(out=xh_t, in_=x_d.ap()[tt * P:(tt + 1) * P, :])
                    for n2 in range(2):
                        ps_sq = psSQ.tile([P, 512], F32, tag="psq")
                        nc.tensor.matmul(
                            ps_sq,
                            wproj_sb[:, tt * P:(tt + 1) * P],
                            attn_sb[:, n2 * 512:(n2 + 1) * 512],
                            start=True, stop=True,
                        )
                        nc.vector.tensor_add(
                            xres[:, tt, n2 * 512:(n2 + 1) * 512], ps_sq,
                            xh_t[:, n2 * 512:(n2 + 1) * 512])
                    st2 = stat2.tile([P, 2, 6], F32, tag="st2")
                    xg2 = xres[:, tt, :].rearrange("p (g d) -> p g d", g=2)
                    for g in range(2):
                        nc.vector.bn_stats(out=st2[:, g, :], in_=xg2[:, g, :])
                    mv2 = stat2.tile([P, 2], F32, tag="mv2")
                    nc.vector.bn_aggr(out=mv2, in_=st2)
                    std2 = stat2.tile([P, 1], F32, tag="std2")
                    nc.scalar.activation(std2, mv2[:, 1:2], AF.Sqrt, bias=epsS_t,
                                         scale=1.0 / 16.0)
                    rstd4b = stat2.tile([P, 1], F32, tag="rstd2")
                    nc.vector.reciprocal(rstd4b, std2)
                    # xn2 = (x-m)*rstd = x*rstd + (-m*rstd): Act Copy w/ scale+bias
                    bneg = stat2.tile([P, 1], F32, tag="bneg")
                    nc.vector.tensor_scalar(
                        out=bneg, in0=mv2[:, 0:1], scalar1=rstd4b, scalar2=-1.0,
                        op0=OP.mult, op1=OP.mult,
                    )
                    xn2_t = x2st.tile([P, E], BF, tag="xn2")
                    nc.scalar.activation(xn2_t, xres[:, tt, :], AF.Identity,
                                         scale=rstd4b, bias=bneg)
                    for eh in range(ET // 4):
                        ps_t2 = psT3.tile([P, 4 * P], BF, tag="pst")
                        for j in range(4):
                            et = eh * 4 + j
                            nc.tensor.transpose(
                                ps_t2[:, j * P:(j + 1) * P],
                                xn2_t[:, et * P:(et + 1) * P], ident,
                            )
                        nc.scalar.activation(
                            h2T[:, eh * 4:(eh + 1) * 4, tt * P:(tt + 1) * P],
                            ps_t2.rearrange("p (a b) -> p a b", a=4), AF.Copy,
                        )

            if phases == "ABC1":
                with tc.tile_pool(name="dbg", bufs=3) as dbg:
                    for tt in range(ST):
                        d_t = dbg.tile([P, E], BF, tag="d")
                        nc.vector.tensor_copy(d_t, xres[:, tt, :])
                        nc.sync.dma_start(out=out_d.ap()[tt * P:(tt + 1) * P, :], in_=d_t)
                return nc

            # ---- C2: FFN, mt-outer so each stationary weight tile is
            # loaded once and reused across all four 512-token chunks ----
            w3_p = fctx.enter_context(tc.tile_pool(name="w3", bufs=1))
            gswp = fctx.enter_context(tc.tile_pool(name="gswp", bufs=1))
            wstream = fctx.enter_context(tc.tile_pool(name="wstream", bufs=2))
            sw = fctx.enter_context(tc.tile_pool(name="sw", bufs=1))
            outs = fctx.enter_context(tc.tile_pool(name="outs", bufs=2))
            psW = fctx.enter_context(tc.tile_pool(name="psW", bufs=1, space="PSUM"))

            w3_sb = w3_p.tile([P, FT, E], F8)
            for k4 in range(FT // 4):
                nc.gpsimd.dma_start(out=w3_sb[:, k4 * 4:(k4 + 1) * 4, :],
                                    in_=w3_d.ap()[:, k4 * 4:(k4 + 1) * 4, :])

            gsw = gswp.tile([P, FT, HSEQ], F8)      # 32*silu(a)*g, full seq
            g_tmp = None
            for mt in range(FT):
                g, mg = mt % 4, mt // 4
                if g == 0:
                    w1_t = wstream.tile([P, 4, ET, P], F8, tag="w1t")
                    nc.sync.dma_start(out=w1_t, in_=w1_d.ap()[mg])
                    w2_t = wstream.tile([P, 4, ET, P], F8, tag="w2t")
                    nc.sync.dma_start(out=w2_t, in_=w2_d.ap()[mg])
                    g_tmp = sw.tile([64, 4, HSEQ], F8, tag="gtmp")
                sw_h = [[None] * NCH, [None] * NCH]
                for mh in range(2):
                    ps_c = []
                    for chk in range(NCH):
                        ps_chk = psW.tile([64, CH], F32, tag=f"pp{mh * NCH + chk}")
                        ps_c.append(ps_chk)
                    for ep in range(EP):
                        for chk in range(NCH):
                            nc.tensor.matmul(
                                ps_c[chk],
                                w1_t[:, g, 2 * ep:2 * ep + 2, mh * 64:mh * 64 + 64],
                                h2T[:, 2 * ep:2 * ep + 2, chk * CH:(chk + 1) * CH],
                                start=(ep == 0), stop=(ep == EP - 1),
                                perf_mode=DR,
                            )
                    for chk in range(NCH):
                        sw_t = sw.tile([64, CH], BF, tag=f"sw{mh}{chk}")
                        if silu_via_sigmoid:
                            sg_t = sw.tile([64, CH], BF, tag=f"sg{mh}{chk}")
                            nc.scalar.activation(sg_t, ps_c[chk], AF.Sigmoid,
                                                 scale=1.0 / 32.0)
                            nc.vector.tensor_scalar_mul(sw_t, ps_c[chk], 1.0 / 32.0)
                            nc.vector.tensor_mul(sw_t, sw_t, sg_t)
                        else:
                            nc.scalar.activation(sw_t, ps_c[chk], AF.Silu,
                                                 scale=1.0 / 32.0)
                        sw_h[mh][chk] = sw_t
                for mh in range(2):
                    ps_c = []
                    for chk in range(NCH):
                        ps_chk = psW.tile([64, CH], F32, tag=f"pp{mh * NCH + chk}")
                        ps_c.append(ps_chk)
                    for ep in range(EP):
                        for chk in range(NCH):
                            nc.tensor.matmul(
                                ps_c[chk],
                                w2_t[:, g, 2 * ep:2 * ep + 2, mh * 64:mh * 64 + 64],
                                h2T[:, 2 * ep:2 * ep + 2, chk * CH:(chk + 1) * CH],
                                start=(ep == 0), stop=(ep == EP - 1),
                                perf_mode=DR,
                            )
                    for chk in range(NCH):
                        if mh == 0:
                            nc.vector.tensor_mul(
                                gsw[0:64, mt, chk * CH:(chk + 1) * CH],
                                ps_c[chk], sw_h[0][chk])
                        else:
                            nc.vector.tensor_mul(
                                g_tmp[:, g, chk * CH:(chk + 1) * CH],
                                ps_c[chk], sw_h[1][chk])
                if g == 3:
                    nc.gpsimd.dma_start(
                        out=gsw[64:P, mg * 4:(mg + 1) * 4, :], in_=g_tmp)

            # ---- W3 + epilogue, one 64-token group at a time; each gsw
            # stationary slice is reused across both 512-col E halves ----
            for t64 in range(2 * ST):
                tt, th = t64 // 2, t64 % 2
                t0 = t64 * 64
                # rotate through the four W1/W2 psum slots (same shape) so
                # consecutive t64 chains double-buffer at no extra banks
                ps_f0 = psW.tile([64, 512], F32, tag=f"pp{(2 * t64) % 8}")
                ps_f1 = psW.tile([64, 512], F32, tag=f"pp{(2 * t64 + 1) % 8}")
                if th == 1:
                    x_odd = outs.tile([64, E], BF, tag="xodd")
                    nc.gpsimd.dma_start(out=x_odd, in_=xres[64:P, tt, :])
                for kp in range(FT // 2):
                    for ec, ps_f in ((0, ps_f0), (1, ps_f1)):
                        nc.tensor.matmul(
                            ps_f,
                            gsw[:, 2 * kp:2 * kp + 2, t0:t0 + 64],
                            w3_sb[:, 2 * kp:2 * kp + 2, ec * 512:(ec + 1) * 512],
                            start=(kp == 0), stop=(kp == FT // 2 - 1),
                            perf_mode=DR,
                        )
                o_t = outs.tile([64, E], BF, tag="ot")
                for ec, ps_f in ((0, ps_f0), (1, ps_f1)):
                    xs = (xres[0:64, tt, ec * 512:(ec + 1) * 512] if th == 0
                          else x_odd[:, ec * 512:(ec + 1) * 512])
                    nc.vector.tensor_add(o_t[:, ec * 512:(ec + 1) * 512],
                                         ps_f, xs)
                nc.sync.dma_start(out=out_d.ap()[t0:t0 + 64, :], in_=o_t)
    return nc


# --------------------------------------------------------------------------
def prep_core_inputs(inputs, core, seq=S):
    """Host-side data prep for one core."""
    b, hf = core // 2, core % 2
    hseq = seq // 2
    ET = E // P
    FT = 4 * E // P
    x = np.asarray(inputs["input_tensor"], np.float32)
    ln1_g = np.asarray(inputs["ln1_g"], np.float32)
    ln1_b = np.asarray(inputs["ln1_b"], np.float32)
    latents = np.asarray(inputs["latents"], np.float32)
    Wq = np.asarray(inputs["Wq"], np.float32)
    Wk = np.asarray(inputs["Wk"], np.float32)
    Wv = np.asarray(inputs["Wv"], np.float32)
    Wo = np.asarray(inputs["Wo"], np.float32)
    bo = np.asarray(inputs["bo"], np.float32)
    Wproj = np.asarray(inputs["Wproj"], np.float32)
    bproj = np.asarray(inputs["bproj"], np.float32)
    ln2_g = np.asarray(inputs["ln2_g"], np.float32)
    ln2_b = np.asarray(inputs["ln2_b"], np.float32)
    W1 = np.asarray(inputs["W1"], np.float32)
    b1 = np.asarray(inputs["b1"], np.float32)
    W2 = np.asarray(inputs["W2"], np.float32)
    b2 = np.asarray(inputs["b2"], np.float32)
    W3 = np.asarray(inputs["W3"], np.float32)
    b3 = np.asarray(inputs["b3"], np.float32)

    assert not (np.any(ln1_b) or np.any(ln2_b) or np.any(bo) or np.any(b1)
                or np.any(b2) or np.any(b3) or np.any(bproj)), \
        "nonzero biases unsupported"

    Wkf = np.transpose(Wk, (1, 0, 2)).reshape(E, H * HD)
    Wvf = np.transpose(Wv, (1, 0, 2)).reshape(E, H * HD)
    wk = (WS * ln1_g[:, None] * Wkf).astype(F8NP).reshape(ET, P, H * HD)
    wk = np.ascontiguousarray(wk.transpose(1, 0, 2))
    wv = (WS * ln1_g[:, None] * Wvf).astype(F8NP).reshape(ET, P, H * HD)
    wv = np.ascontiguousarray(wv.transpose(1, 0, 2))
    wq = Wq.astype(BF16).reshape(H, LD // P, P, HD).transpose(2, 0, 1, 3)
    wq = np.ascontiguousarray(wq).reshape(P, H * (LD // P) * HD)
    lat = latents.transpose(0, 2, 1).astype(BF16)              # [H, LD, NL]
    lat = lat.reshape(H, LD // P, P, NL).transpose(2, 0, 1, 3)
    lat = np.ascontiguousarray(lat).reshape(P, H * (LD // P) * NL)
    wo = np.ascontiguousarray(Wo.astype(BF16).reshape(H, HD, E).transpose(1, 0, 2))
    wproj = np.ascontiguousarray(Wproj[:, hf * hseq:(hf + 1) * hseq].astype(BF16))
    # grouped layout [FT//4, P, 4, ET, P] so 4 hidden tiles load per DMA
    w1 = (WS * ln2_g[:, None] * W1).astype(F8NP).reshape(ET, P, FT // 4, 4, P)
    w1 = np.ascontiguousarray(w1.transpose(2, 1, 3, 0, 4))
    w2 = (WS * ln2_g[:, None] * W2).astype(F8NP).reshape(ET, P, FT // 4, 4, P)
    w2 = np.ascontiguousarray(w2.transpose(2, 1, 3, 0, 4))
    w3 = np.ascontiguousarray((W3S * W3).astype(F8NP).reshape(FT, P, E).transpose(1, 0, 2))
    x_own = np.ascontiguousarray((RS * x[b, hf * hseq:(hf + 1) * hseq]).astype(BF16))
    return {
        "x_own": x_own,
        "wk": wk, "wv": wv, "wq": wq, "lat": lat, "wo": wo, "wproj": wproj,
        "w1": w1, "w2": w2, "w3": w3,
    }


_CACHE = {}


def kernel(**inputs) -> np.ndarray:
    if "nc" not in _CACHE:
        nc = bass.Bass("TRN2", target_bir_lowering=False, debug=False,
                       num_devices=NCORES)
        build_program(nc, seq=S)
        _spill_extra_waits(nc)
        _CACHE["nc"] = nc
    nc = _CACHE["nc"]

    in_maps = [prep_core_inputs(inputs, c) for c in range(NCORES)]
    from concourse.bass_utils import run_bass_kernel_spmd
    res = run_bass_kernel_spmd(nc, in_maps, core_ids=list(range(NCORES)))

    out = np.empty((B, S, E), np.float32)
    for c in range(NCORES):
        b, hf = c // 2, c % 2
        out[b, hf * (S // 2):(hf + 1) * (S // 2)] = (
            res.results[c]["out"].astype(np.float32) / RS)
    return out

